# revision 13
# baseline (speedup 1.0000x reference)
"""GQA cross-attention block on 8 trn2 NeuronCores — v4.

Sharding: tensor-parallel over heads. Core c owns KV group g=c and its 4
query heads; it computes its heads' attention plus its 256-row slice of
the o-projection, producing a full-shape partial output; the host sums
the 8 partials and adds bo' = bo + bv_expanded @ Wo (the v bias folds
through softmax's row-sum-1 into a constant, so v needs no on-chip bias).

Attention structure (v2): all attention matmuls in (64,128) row-tiled
mode so the PE HAM clock-gate stays at 2.4GHz; k is duplicated on both
partition halves ([wk|wk] stationary) so head A/B scores run
concurrently as row-tiles T0/T8 into one [128,1024] PSUM tile consumed
by a single exp ACTIVATE (amortizes ScalarE's 352-cycle overhead); AV
is K-split with a ones-column giving the softmax denominator Z.

v3 -> v4 (trace-driven):
  - queue remap: GpSimd runs only the attention-critical partition
    broadcasts + memsets; input streaming DMAs moved to the Sync queue;
    va DMA-transposes to the Scalar queue; per-row [128,2048] output
    DMAs on Sync.
  - batch 0's head shrinks: k/v projections (which gate all of
    attention) run first, q is projected st0/st1 up front and st2/st3
    as same-mode blocks between early attention qc groups, so attention
    starts as soon as k/v and one q tile exist.
  - o-proj PSUM evacuation alternates Scalar/Vector so neither engine
    paces the merged o-proj(b) + proj(b+1) window.
"""

import numpy as np
import ml_dtypes

import concourse.bass as bass
from concourse import bacc
import concourse.mybir as mybir
import concourse.tile as tile
from concourse.bass_utils import run_bass_kernel_spmd

BF16 = ml_dtypes.bfloat16
F32 = mybir.dt.float32
BF = mybir.dt.bfloat16

B = 2
S = 2048
HID = 2048
D = 64          # head dim
NCORES = 8
NH = HID // 128  # 16 hidden chunks
NST = S // 512   # 4 s-tiles of 512
NKC = S // 128   # 16 key chunks of 128
SCALE = 1.0 / np.sqrt(D)

ID = mybir.ActivationFunctionType.Identity
EXP = mybir.ActivationFunctionType.Exp


def _build_nc() -> bass.Bass:
    nc = bacc.Bacc()

    xT = nc.dram_tensor("xT", [B, HID, S], BF, kind="ExternalInput")
    encT = nc.dram_tensor("encT", [B, HID, S], BF, kind="ExternalInput")
    wq = nc.dram_tensor("wq", [HID, 256], BF, kind="ExternalInput")
    wkk = nc.dram_tensor("wkk", [HID, 128], BF, kind="ExternalInput")
    wvv = nc.dram_tensor("wvv", [HID, 128], BF, kind="ExternalInput")
    wo = nc.dram_tensor("wo", [256, HID], BF, kind="ExternalInput")
    bq = nc.dram_tensor("bq", [256, 1], F32, kind="ExternalInput")
    bkk = nc.dram_tensor("bkk", [128, 1], F32, kind="ExternalInput")
    out = nc.dram_tensor("out", [B, S, HID], BF, kind="ExternalOutput")

    with tile.TileContext(nc) as tc:
        with (
            tc.tile_pool(name="wpool", bufs=1) as wpool,
            tc.tile_pool(name="xs", bufs=18) as xs_pool,
            tc.tile_pool(name="es", bufs=18) as es_pool,
            tc.tile_pool(name="acts", bufs=2) as acts,
            tc.tile_pool(name="vpool", bufs=2) as vpool,
            tc.tile_pool(name="epool", bufs=4) as epool,
            tc.tile_pool(name="nrm", bufs=3) as nrm,
            tc.tile_pool(name="osb", bufs=4) as osb_pool,
            tc.tile_pool(name="psq", bufs=2, space="PSUM") as psq,
            tc.tile_pool(name="psw", bufs=4, space="PSUM") as psw,
        ):
            # ---- resident weights ----
            wq_t = []
            wkk_t = []
            wvv_t = []
            for h in range(NH):
                hsl = slice(h * 128, (h + 1) * 128)
                wqh = wpool.tile([128, 256], BF, name=f"wq{h}")
                nc.sync.dma_start(out=wqh[:], in_=wq[hsl, :])
                wq_t.append(wqh)
                wkh = wpool.tile([128, 128], BF, name=f"wkk{h}")
                nc.sync.dma_start(out=wkh[:], in_=wkk[hsl, :])
                wkk_t.append(wkh)
                wvh = wpool.tile([128, 128], BF, name=f"wvv{h}")
                nc.sync.dma_start(out=wvh[:], in_=wvv[hsl, :])
                wvv_t.append(wvh)
            woAB = wpool.tile([128, HID], BF, name="woAB")
            nc.sync.dma_start(out=woAB[:], in_=wo[0:128, :])
            woCD = wpool.tile([128, HID], BF, name="woCD")
            nc.sync.dma_start(out=woCD[:], in_=wo[128:256, :])
            bqAB = wpool.tile([128, 1], F32, name="bqAB")
            nc.sync.dma_start(out=bqAB[:], in_=bq[0:128, :])
            bqCD = wpool.tile([128, 1], F32, name="bqCD")
            nc.sync.dma_start(out=bqCD[:], in_=bq[128:256, :])
            bkk_t = wpool.tile([128, 1], F32, name="bkk_t")
            nc.sync.dma_start(out=bkk_t[:], in_=bkk[:, :])

            st_tiles = {}

            def get_acts(b):
                if b not in st_tiles:
                    st_tiles[b] = {
                        "qAB": acts.tile([128, S], BF, tag="qAB", name=f"qAB{b}"),
                        "qCD": acts.tile([128, S], BF, tag="qCD", name=f"qCD{b}"),
                        "kdup": acts.tile([128, S], BF, tag="kdup",
                                          name=f"kdup{b}"),
                        "vT": acts.tile([128, S], BF, tag="vT", name=f"vT{b}"),
                        "oAB": acts.tile([128, S], BF, tag="oAB", name=f"oAB{b}"),
                        "oCD": acts.tile([128, S], BF, tag="oCD", name=f"oCD{b}"),
                        "va": [],
                        "xt": {},
                        "et": {},
                    }
                return st_tiles[b]

            def emit_et_dma(b, stp):
                t = get_acts(b)
                psl = slice(stp * 1024, (stp + 1) * 1024)
                tiles = []
                for h in range(NH):
                    hsl = slice(h * 128, (h + 1) * 128)
                    et = es_pool.tile([128, 1024], BF, tag="es",
                                      name=f"es{b}{stp}{h}")
                    nc.gpsimd.dma_start(out=et[:], in_=encT[b, hsl, psl])
                    tiles.append(et)
                t["et"][stp] = tiles

            def emit_xt_dma(b, stp):
                t = get_acts(b)
                psl = slice(stp * 1024, (stp + 1) * 1024)
                tiles = []
                for h in range(NH):
                    hsl = slice(h * 128, (h + 1) * 128)
                    xt = xs_pool.tile([128, 1024], BF, tag="xs",
                                      name=f"xs{b}{stp}{h}")
                    nc.gpsimd.dma_start(out=xt[:], in_=xT[b, hsl, psl])
                    tiles.append(xt)
                t["xt"][stp] = tiles

            def emit_kv_half(b, stp):
                """k/v projections for s-tiles 2*stp, 2*stp+1 (needs et)."""
                t = get_acts(b)
                et_t = t["et"][stp]
                for sub in range(2):
                    st = 2 * stp + sub
                    ssl = slice(st * 512, (st + 1) * 512)
                    msl = slice(sub * 512, (sub + 1) * 512)
                    kkps = psw.tile([128, 512], F32, tag="work",
                                    name=f"kkps{b}{st}")
                    vvps = psw.tile([128, 512], F32, tag="work",
                                    name=f"vvps{b}{st}")
                    for h in range(NH):
                        nc.tensor.matmul(
                            kkps[:], wkk_t[h][:], et_t[h][:, msl],
                            start=(h == 0), stop=(h == NH - 1))
                        nc.tensor.matmul(
                            vvps[:], wvv_t[h][:], et_t[h][:, msl],
                            start=(h == 0), stop=(h == NH - 1))
                    nc.scalar.activation(t["kdup"][:, ssl], kkps[:], ID,
                                         bias=bkk_t[:])
                    nc.vector.tensor_copy(t["vT"][0:64, ssl], vvps[0:64, :])

            def emit_q_st(b, st):
                """q projection for one s-tile (needs xt half st//2)."""
                t = get_acts(b)
                xt_t = t["xt"][st // 2]
                ssl = slice(st * 512, (st + 1) * 512)
                msl = slice((st % 2) * 512, (st % 2 + 1) * 512)
                qps_lo = psw.tile([128, 512], F32, tag="work",
                                  name=f"qpl{b}{st}")
                qps_hi = psw.tile([128, 512], F32, tag="work",
                                  name=f"qph{b}{st}")
                for h in range(NH):
                    nc.tensor.matmul(
                        qps_lo[:], wq_t[h][:, 0:128], xt_t[h][:, msl],
                        start=(h == 0), stop=(h == NH - 1))
                    nc.tensor.matmul(
                        qps_hi[:], wq_t[h][:, 128:256], xt_t[h][:, msl],
                        start=(h == 0), stop=(h == NH - 1))
                nc.scalar.activation(t["qAB"][:, ssl], qps_lo[:], ID,
                                     bias=bqAB[:])
                nc.scalar.activation(t["qCD"][:, ssl], qps_hi[:], ID,
                                     bias=bqCD[:])

            def emit_va(b, half):
                """v_aug transposes for one kv half (Scalar queue)."""
                t = get_acts(b)
                for kc in range(half * 8, half * 8 + 8):
                    vak = vpool.tile([128, 66], BF, tag=f"va{kc}",
                                     name=f"va{b}{kc}")
                    nc.gpsimd.memset(vak[:, 64:65], 1.0)
                    nc.scalar.dma_start_transpose(
                        vak[:, 0:64], t["vT"][0:64, kc * 128:(kc + 1) * 128])
                    t["va"].append(vak)

            def emit_attention_qc(b, qc):
                t = get_acts(b)
                va = t["va"]
                qsl = slice(qc * 512, (qc + 1) * 512)
                for qsrc, odst, pname in ((t["qAB"], t["oAB"], "AB"),
                                          (t["qCD"], t["oCD"], "CD")):
                    avs = [
                        psw.tile([128, 512], F32, tag="work",
                                 name=f"av{b}{pname}{qc}{i}")
                        for i in range(4)
                    ]
                    eqs = [None] * NKC

                    def emit_av(kc):
                        eq = eqs[kc]
                        st0 = (kc == 0)
                        sp0 = (kc == NKC - 1)
                        nc.tensor.matmul(
                            avs[0][0:65, :], va[kc][0:64, 0:65],
                            eq[0:64, 0:512], start=st0, stop=sp0)
                        nc.tensor.matmul(
                            avs[1][0:65, :], va[kc][64:128, 0:65],
                            eq[64:128, 0:512], start=st0, stop=sp0)
                        nc.tensor.matmul(
                            avs[2][0:65, :], va[kc][0:64, 0:65],
                            eq[0:64, 512:1024], start=st0, stop=sp0)
                        nc.tensor.matmul(
                            avs[3][0:65, :], va[kc][64:128, 0:65],
                            eq[64:128, 512:1024], start=st0, stop=sp0)

                    for kc in range(NKC):
                        ksl = slice(kc * 128, (kc + 1) * 128)
                        sq = psq.tile([128, 1024], F32, tag="sq",
                                      name=f"sq{b}{pname}{qc}{kc}")
                        nc.tensor.matmul(
                            sq[:, 0:512], t["kdup"][0:64, ksl],
                            qsrc[0:64, qsl], start=True, stop=True)
                        nc.tensor.matmul(
                            sq[:, 512:1024], t["kdup"][64:128, ksl],
                            qsrc[64:128, qsl], start=True, stop=True)
                        eq = epool.tile([128, 1024], BF, tag="eq",
                                        name=f"eq{b}{pname}{qc}{kc}")
                        nc.scalar.activation(eq[:], sq[:], EXP,
                                             scale=float(SCALE))
                        eqs[kc] = eq
                        if kc > 0:
                            emit_av(kc - 1)
                    emit_av(NKC - 1)

                    for hh in range(2):
                        avo = nrm.tile([128, 512], F32, tag="avo",
                                       name=f"avo{b}{pname}{qc}{hh}")
                        nc.vector.tensor_copy(
                            avo[0:65, :], avs[2 * hh + 1][0:65, :])
                        avsb = nrm.tile([128, 512], F32, tag="avsb",
                                        name=f"avsb{b}{pname}{qc}{hh}")
                        nc.vector.tensor_add(
                            avsb[0:65, :], avs[2 * hh][0:65, :],
                            avo[0:65, :])
                        rz = nrm.tile([128, 512], F32, tag="rz",
                                      name=f"rz{b}{pname}{qc}{hh}")
                        nc.vector.reciprocal(rz[0:1, :], avsb[64:65, :])
                        rb = nrm.tile([128, 512], F32, tag="rb",
                                      name=f"rb{b}{pname}{qc}{hh}")
                        nc.gpsimd.partition_broadcast(rb[0:64, :], rz[0:1, :])
                        nc.vector.tensor_mul(
                            odst[hh * 64:(hh + 1) * 64, qsl],
                            avsb[0:64, :], rb[0:64, :])

            def emit_oproj(b, sc_lo, sc_hi):
                # ops tiles live in the sq (psq) banks, which are idle
                # between attention windows; [128,1024] pairs halve the
                # evacuation instruction count, split across ScE/DVE.
                t = get_acts(b)
                for sc in range(sc_lo, sc_hi):
                    s128 = slice(sc * 128, (sc + 1) * 128)
                    ob = osb_pool.tile([128, HID], BF, tag="osb",
                                       name=f"ob{b}{sc}")
                    for hp in range(2):
                        ops = psq.tile([128, 1024], F32, tag="sq",
                                       name=f"ops{b}{sc}{hp}")
                        for sub in range(2):
                            hc = 2 * hp + sub
                            hsl = slice(hc * 512, (hc + 1) * 512)
                            osl = slice(sub * 512, (sub + 1) * 512)
                            nc.tensor.matmul(
                                ops[:, osl], t["oAB"][:, s128], woAB[:, hsl],
                                start=True, stop=False)
                            nc.tensor.matmul(
                                ops[:, osl], t["oCD"][:, s128], woCD[:, hsl],
                                start=False, stop=True)
                        hpsl = slice(hp * 1024, (hp + 1) * 1024)
                        if hp == 0:
                            nc.vector.tensor_copy(ob[:, hpsl], ops[:])
                        else:
                            nc.scalar.copy(ob[:, hpsl], ops[:])
                    nc.sync.dma_start(out=out[b, s128, :], in_=ob[:])

            # ---- schedule ----
            # b0 head: enc half0, x half0, enc half1 — so k/v and q st0
            # projections start as early as their transfers land
            emit_et_dma(0, 0)
            emit_xt_dma(0, 0)
            emit_et_dma(0, 1)
            emit_kv_half(0, 0)
            emit_va(0, 0)
            emit_kv_half(0, 1)
            emit_va(0, 1)
            emit_q_st(0, 0)
            emit_q_st(0, 1)
            for b in range(B):
                emit_attention_qc(b, 0)
                emit_xt_dma(b, 1)        # prefetch q st2/st3 inputs
                emit_attention_qc(b, 1)
                emit_q_st(b, 2)
                emit_attention_qc(b, 2)
                emit_q_st(b, 3)
                emit_attention_qc(b, 3)
                if b + 1 < B:
                    # merged window: b's o-proj first (no deps on b+1);
                    # b+1's enc/x stream in on the gpsimd queue meanwhile
                    emit_et_dma(b + 1, 0)
                    emit_xt_dma(b + 1, 0)
                    emit_et_dma(b + 1, 1)
                    emit_oproj(b, 0, 8)
                    emit_kv_half(b + 1, 0)
                    emit_va(b + 1, 0)
                    emit_oproj(b, 8, 16)
                    emit_kv_half(b + 1, 1)
                    emit_va(b + 1, 1)
                    emit_q_st(b + 1, 0)
                    emit_q_st(b + 1, 1)
                else:
                    emit_oproj(b, 0, 16)

    if not nc.is_finalized():
        nc.finalize()
    return nc


_NC = None
_RUN_KWARGS = {}
_LAST_RESULT = None


def _get_nc():
    global _NC
    if _NC is None:
        _NC = _build_nc()
    return _NC


def kernel(x, encoder_output, Wq, bq, Wk, bk, Wv, bv, Wo, bo):
    nc = _get_nc()
    xT = np.ascontiguousarray(
        np.asarray(x, np.float32).transpose(0, 2, 1)).astype(BF16)
    encT = np.ascontiguousarray(
        np.asarray(encoder_output, np.float32).transpose(0, 2, 1)).astype(BF16)
    Wq = np.asarray(Wq, np.float32)
    Wk = np.asarray(Wk, np.float32)
    Wv = np.asarray(Wv, np.float32)
    Wo = np.asarray(Wo, np.float32)
    bq_f = np.asarray(bq, np.float32)
    bk_f = np.asarray(bk, np.float32)
    bv_f = np.asarray(bv, np.float32)
    bo_f = np.asarray(bo, np.float32)

    in_maps = []
    for c in range(NCORES):
        csl = slice(c * 256, (c + 1) * 256)
        gsl = slice(c * D, (c + 1) * D)
        wk_c = Wk[:, gsl]
        wv_c = Wv[:, gsl]
        in_maps.append({
            "xT": xT,
            "encT": encT,
            "wq": np.ascontiguousarray(Wq[:, csl]).astype(BF16),
            "wkk": np.ascontiguousarray(
                np.concatenate([wk_c, wk_c], axis=1)).astype(BF16),
            "wvv": np.ascontiguousarray(
                np.concatenate([wv_c, wv_c], axis=1)).astype(BF16),
            "wo": np.ascontiguousarray(Wo[csl, :]).astype(BF16),
            "bq": np.ascontiguousarray(bq_f[csl].reshape(256, 1)),
            "bkk": np.ascontiguousarray(
                np.concatenate([bk_f[gsl], bk_f[gsl]]).reshape(128, 1)),
        })
    res = run_bass_kernel_spmd(nc, in_maps, list(range(NCORES)), **_RUN_KWARGS)
    global _LAST_RESULT
    _LAST_RESULT = res
    total = np.zeros((B, S, HID), np.float32)
    for c in range(NCORES):
        total += res.results[c]["out"].astype(np.float32)
    # bv folds through softmax (rows sum to 1) into a constant output bias
    bv_exp = np.repeat(bv_f.reshape(NCORES, 1, D), 4, axis=1).reshape(-1)
    return total + bo_f + bv_exp @ Wo
